# revision 1
# baseline (speedup 1.0000x reference)
"""FAGCN forward on 8 TRN2 NeuronCores (Bass/Tile).

Sharding: row-partition of nodes, 8 ways. The dense input projection
(h = relu(x @ t1^T + b)) is replicated on every core into a 512B-stride
gather table [h bf16 x128 | b f32 | pad] (prep batched 8 chunks/group).
Per layer the edge phase is a two-stream token walk ([all-lo windows]
[all-hi windows], int16 gather indices split at lo_split): 1024-edge
dma_gather calls (small calls + 4 G buffers let the Q7 desc-gen
pipeline two calls at once — the binding resource) fetch source rows.
The gate's a[row] term is recovered on TensorE: a host-precomputed
transposed one-hot (eht, streamed via plain DMA) is the stationary of a
[tok,1] matmul against the per-window a vector; one tanh per call turns
a+b into edge weights. The scatter-add is a per-tile fused
(iota==rr)*w one-hot (single DVE op) feeding a TensorE matmul into a
per-window PSUM accumulator; windows accumulate in SBUF across the two
streams. Between layers the owned rows are AllGathered. The head
(t2 matmul + log_softmax) runs as a final two-pass sweep so the Exp/Ln
activation tables load once each.
"""

import os
import sys
import numpy as np

sys.path.insert(0, "/opt/trn_rl_repo")

import concourse.bass as bass
import concourse.bacc as bacc
import concourse.mybir as mybir
import concourse.tile as tile
from concourse import library_config

F32 = mybir.dt.float32
BF16 = mybir.dt.bfloat16
I16 = mybir.dt.int16

# problem constants (self-contained per contract)
N_NODES = 50000
IN_CH = 256
HIDDEN = 128
OUT_CH = 64
EPS = 0.3
NCORES = 8
CALL_TOKENS = int(os.environ.get("KCT", "1024"))
CT_MAX = CALL_TOKENS // 128
EXT_SLOTS = 128   # 512B gather record
B_SLOT = 64       # f32 slot holding the gate b-term
PREP_GRP = 8


def _install_profile_hook():
    import types
    name = "antenv.axon_hooks"
    if name in sys.modules:
        return
    try:
        import trn_agent_boot.trn_boot as tb
        hook = tb._ntff_profile_via_ctypes("/opt/axon/libaxon_pjrt.so")
    except Exception:
        hook = None
    mod = types.ModuleType(name)
    mod._hook = hook
    mod.get_axon_ntff_profile_hook = lambda: mod._hook
    mod.set_axon_ntff_profile_hook = lambda h: setattr(mod, "_hook", h)
    sys.modules[name] = mod


# ======================================================================
# Host preprocessing: SPMD token streams + per-core data
# ======================================================================

def preprocess(edge_index, n_nodes, ncores, lo_split):
    row = np.asarray(edge_index[0], dtype=np.int64)
    col = np.asarray(edge_index[1], dtype=np.int64)
    E = row.shape[0]
    r_per = n_nodes // ncores
    nwin = (r_per + 127) // 128

    deg = np.bincount(row, minlength=n_nodes).astype(np.float64)
    dinv = np.where(deg > 0, 1.0 / np.sqrt(np.maximum(deg, 1.0)), 0.0)
    escale_all = (dinv[row] * dinv[col]).astype(np.float32)

    core = row // r_per
    lrow = row - core * r_per
    win = lrow // 128
    is_hi = (col >= lo_split).astype(np.int64)

    # stream order: core, then stream (lo/hi), then window, then lrow
    order = np.lexsort((lrow, win, is_hi, core))
    core_s, win_s, hi_s = core[order], win[order], is_hi[order]
    lrow_s, col_s, esc_s = lrow[order], col[order], escale_all[order]

    key = (core_s * 2 + hi_s) * nwin + win_s
    cnt = np.bincount(key, minlength=ncores * 2 * nwin).reshape(ncores, 2, nwin)
    sec_len = ((cnt.max(axis=0) + 127) // 128) * 128  # [2, nwin]
    L_lo = int(sec_len[0].sum())
    L_hi = int(sec_len[1].sum())
    e_tok = L_lo + L_hi
    sec_start = np.zeros((2, nwin), np.int64)
    sec_start[0] = np.concatenate([[0], np.cumsum(sec_len[0])[:-1]])
    sec_start[1] = L_lo + np.concatenate([[0], np.cumsum(sec_len[1])[:-1]])

    col16 = np.zeros((ncores, e_tok), np.int16)
    rowrel = np.zeros((ncores, e_tok), np.float32)
    esc = np.zeros((ncores, e_tok), np.float32)

    grp_first = np.zeros(ncores * 2 * nwin + 1, np.int64)
    np.cumsum(cnt.reshape(-1), out=grp_first[1:])
    rank = np.arange(E) - grp_first[key]
    dest = sec_start[hi_s, win_s] + rank
    cval = np.where(hi_s == 1, col_s - lo_split, col_s).astype(np.int16)
    col16[core_s, dest] = cval
    rowrel[core_s, dest] = (lrow_s - win_s * 128).astype(np.float32)
    esc[core_s, dest] = esc_s

    # gather calls per stream
    calls = []  # (stream, ts, nt)
    for h, base, L in ((0, 0, L_lo), (1, L_lo, L_hi)):
        off = 0
        while off < L:
            nt = min(CALL_TOKENS, L - off)
            calls.append((h, base + off, nt))
            off += nt

    idx_dev = np.zeros((ncores, 128, e_tok // 16), np.int16)
    for (h, ts, nt) in calls:
        blk = col16[:, ts:ts + nt].reshape(ncores, nt // 16, 16)
        blk = np.ascontiguousarray(np.transpose(blk, (0, 2, 1)))
        idx_dev[:, :, ts // 16:(ts + nt) // 16] = np.tile(blk, (1, 8, 1))
    rr_dev = np.ascontiguousarray(rowrel.reshape(ncores, -1, 128).transpose(0, 2, 1))
    es_dev = np.ascontiguousarray(esc.reshape(ncores, -1, 128).transpose(0, 2, 1))

    # transposed one-hot, partition-major: eht[r, i] = 1 iff rowrel[i] == r
    import ml_dtypes
    eht = np.zeros((ncores, 128 * e_tok), ml_dtypes.bfloat16)
    ii = np.arange(e_tok)
    for c in range(ncores):
        eht[c][rowrel[c].astype(np.int64) * e_tok + ii] = 1.0
    eht = eht.reshape(ncores, 128, e_tok)

    return {
        "nwin": nwin, "e_tok": e_tok, "sec_len": sec_len, "calls": calls,
        "idx_dev": idx_dev, "rr_dev": rr_dev, "es_dev": es_dev, "eht_dev": eht,
    }


# ======================================================================
# Kernel builder
# ======================================================================

def build_kernel(meta, n_nodes, in_ch, hidden, out_ch, eps, lo_split, ncores):
    nwin = meta["nwin"]
    e_tok = meta["e_tok"]
    sec_len = meta["sec_len"]
    calls = meta["calls"]
    nchunk_tot = (n_nodes + 127) // 128
    r_per = n_nodes // ncores
    last_win_rows = r_per - 128 * (nwin - 1)
    kt = in_ch // 128
    hh = hidden // 2  # f32 slots holding the bf16 h vector

    # tile -> window map, and burst boundaries per (stream, window)
    tiles_w = []
    burst = {}  # (h, w) -> (gfirst, glast) in global tile idx
    for h in range(2):
        for w in range(nwin):
            ntl = int(sec_len[h, w]) // 128
            if ntl == 0:
                continue
            g0 = len(tiles_w)
            tiles_w.extend([w] * ntl)
            burst[(h, w)] = (g0, g0 + ntl - 1)
    assert len(tiles_w) == e_tok // 128
    last_stream = {}
    for w in range(nwin):
        last_stream[w] = 1 if (1, w) in burst else 0

    ngrp = nchunk_tot // PREP_GRP
    grp_rem = nchunk_tot - ngrp * PREP_GRP
    ogrp = nwin // PREP_GRP
    ogrp_rem = nwin - ogrp * PREP_GRP

    nc = bacc.Bacc("TRN2", target_bir_lowering=False, debug=False,
                   num_devices=ncores, num_swdge_queues=4)

    # ---- I/O ----
    # xtg: host-prearranged [group, 128p, grp*kt, 128] bf16 (+ ones row separately)
    xtg = nc.dram_tensor("xtg", [ngrp + (1 if grp_rem else 0), 128, PREP_GRP * kt, 128], BF16, kind="ExternalInput")
    xbg = nc.dram_tensor("xbg", [ngrp + (1 if grp_rem else 0), 1, PREP_GRP, 128], BF16, kind="ExternalInput")
    xtog = nc.dram_tensor("xtog", [ogrp + (1 if ogrp_rem else 0), 128, PREP_GRP * kt, 128], BF16, kind="ExternalInput")
    xbog = nc.dram_tensor("xbog", [ogrp + (1 if ogrp_rem else 0), 1, PREP_GRP, 128], BF16, kind="ExternalInput")
    t1wt = nc.dram_tensor("t1wt", [in_ch + 1, hidden], BF16, kind="ExternalInput")
    gwrep = nc.dram_tensor("gwrep", [4, 128, hidden], BF16, kind="ExternalInput")
    gbrep = nc.dram_tensor("gbrep", [128, 2], F32, kind="ExternalInput")
    t2wt = nc.dram_tensor("t2wt", [hidden, out_ch], F32, kind="ExternalInput")
    t2b = nc.dram_tensor("t2b", [1, out_ch], F32, kind="ExternalInput")
    iota_in = nc.dram_tensor("iota", [128, 128], BF16, kind="ExternalInput")
    ident_in = nc.dram_tensor("ident", [128, 128], F32, kind="ExternalInput")
    ones_in = nc.dram_tensor("ones", [1, 128], F32, kind="ExternalInput")
    idx_in = nc.dram_tensor("idx", [128, e_tok // 16], I16, kind="ExternalInput")
    eht_in = nc.dram_tensor("eht", [128, e_tok], BF16, kind="ExternalInput")
    rr_in = nc.dram_tensor("rr", [128, e_tok // 128], F32, kind="ExternalInput")
    es_in = nc.dram_tensor("es", [128, e_tok // 128], F32, kind="ExternalInput")
    out = nc.dram_tensor("out", [r_per, out_ch], F32, kind="ExternalOutput")

    ext0 = nc.dram_tensor("ext0", [nchunk_tot * 128, EXT_SLOTS], F32)
    agi = nc.dram_tensor("agi", [r_per, EXT_SLOTS], F32)
    ago = nc.dram_tensor("ago", [r_per * ncores, EXT_SLOTS], F32)

    with tile.TileContext(nc) as tc:
        nc.gpsimd.load_library(library_config.mlp)
        with tc.tile_pool(name="consts", bufs=1) as cp:
            t1wt_sb = cp.tile([128, kt, hidden], BF16, tag="t1wt")
            nc.sync.dma_start(t1wt_sb[:], bass.AP(t1wt, 0, [[hidden, 128], [128 * hidden, kt], [1, hidden]]))
            t1b_sb = cp.tile([1, hidden], BF16, tag="t1b")
            nc.sync.dma_start(t1b_sb[:], t1wt.ap()[in_ch:in_ch + 1, :])
            gw_sb = cp.tile([128, 4, hidden], BF16, tag="gw")
            nc.sync.dma_start(gw_sb[:], bass.AP(gwrep, 0, [[hidden, 128], [128 * hidden, 4], [1, hidden]]))
            gb_sb = cp.tile([128, 2], F32, tag="gb")
            nc.sync.dma_start(gb_sb[:], gbrep.ap())
            t2wt_sb = cp.tile([128, out_ch], F32, tag="t2wt")
            nc.sync.dma_start(t2wt_sb[:], t2wt.ap())
            t2b_sb = cp.tile([1, out_ch], F32, tag="t2b")
            nc.sync.dma_start(t2b_sb[:], t2b.ap())
            iota_sb = cp.tile([128, 128], BF16, tag="iota")
            nc.sync.dma_start(iota_sb[:], iota_in.ap())
            ident_sb = cp.tile([128, 128], F32, tag="ident")
            nc.sync.dma_start(ident_sb[:], ident_in.ap())
            ones_sb = cp.tile([1, 128], F32, tag="ones")
            nc.sync.dma_start(ones_sb[:], ones_in.ap())
            idxt = cp.tile([128, e_tok // 16], I16, tag="idxt")
            nc.sync.dma_start(idxt[:], idx_in.ap())
            rr_sb = cp.tile([128, e_tok // 128], F32, tag="rr")
            nc.sync.dma_start(rr_sb[:], rr_in.ap())
            es_sb = cp.tile([128, e_tok // 128], F32, tag="es")
            nc.sync.dma_start(es_sb[:], es_in.ap())

            rawsc = cp.tile([128, nwin, hidden], F32, tag="rawsc")
            acc = cp.tile([128, nwin, hidden], F32, tag="acc")
            a_arr = cp.tile([128, nwin, 2], BF16, tag="a_arr")

            # ---------------- prep: replicated gather table ----------------
            with tc.tile_pool(name="prep", bufs=3) as pp, \
                 tc.tile_pool(name="prep_s", bufs=6) as pscr, \
                 tc.tile_pool(name="prep_ps", bufs=4, space="PSUM") as pps:

                def prep_group(gi, gcnt, xt_t, xb_t, own):
                    xt_sb = pp.tile([128, PREP_GRP * kt, 128], BF16, tag="xt")
                    nc.sync.dma_start(xt_sb[:, 0:gcnt * kt, :], xt_t.ap()[gi, :, 0:gcnt * kt, :])
                    xb_sb = pp.tile([1, PREP_GRP, 128], BF16, tag="xb")
                    nc.sync.dma_start(xb_sb[:, 0:gcnt, :], xb_t.ap()[gi, :, 0:gcnt, :])
                    if not own:
                        extg = pp.tile([128, PREP_GRP, B_SLOT + 1], F32, tag="extg")
                    for c in range(gcnt):
                        ps = pps.tile([128, hidden], F32, tag="h0ps")
                        for k in range(kt):
                            nc.tensor.matmul(ps[:], xt_sb[:, c * kt + k, :], t1wt_sb[:, k, :],
                                             start=(k == 0), stop=False)
                        nc.tensor.matmul(ps[:], xb_sb[:, c, :], t1b_sb[:], start=False, stop=True)
                        if own:
                            w = gi * PREP_GRP + c
                            nc.vector.tensor_scalar(out=rawsc[:, w, :], in0=ps[:],
                                                    scalar1=0.0, scalar2=eps,
                                                    op0=mybir.AluOpType.max,
                                                    op1=mybir.AluOpType.mult)
                            hb = pscr.tile([128, hidden], BF16, tag="hb")
                            nc.scalar.activation(hb[:], ps[:], mybir.ActivationFunctionType.Relu)
                            scr = pscr.tile([128, hidden], BF16, tag="scr")
                            a_f = pscr.tile([128, 1], F32, tag="af")
                            nc.vector.scalar_tensor_tensor(
                                out=scr[:], in0=hb[:], scalar=1.0, in1=gw_sb[:, 0, :],
                                op0=mybir.AluOpType.mult, op1=mybir.AluOpType.mult,
                                accum_out=a_f[:])
                            nc.vector.tensor_scalar(out=a_arr[:, w, 0:1], in0=a_f[:],
                                                    scalar1=gb_sb[:, 0:1], scalar2=None,
                                                    op0=mybir.AluOpType.add)
                        else:
                            hb = extg[:, c, 0:hh].bitcast(BF16)
                            nc.scalar.activation(hb, ps[:], mybir.ActivationFunctionType.Relu)
                            scr = pscr.tile([128, hidden], BF16, tag="scr")
                            nc.vector.scalar_tensor_tensor(
                                out=scr[:], in0=hb, scalar=1.0, in1=gw_sb[:, 1, :],
                                op0=mybir.AluOpType.mult, op1=mybir.AluOpType.mult,
                                accum_out=extg[:, c, B_SLOT:B_SLOT + 1])
                    if not own:
                        base = gi * PREP_GRP * 128
                        nc.sync.dma_start(
                            bass.AP(ext0, base * EXT_SLOTS,
                                    [[EXT_SLOTS, 128], [128 * EXT_SLOTS, gcnt], [1, B_SLOT + 1]]),
                            extg[:, 0:gcnt, :])

                for gi in range(ngrp + (1 if grp_rem else 0)):
                    prep_group(gi, PREP_GRP if gi < ngrp else grp_rem, xtg, xbg, False)
                for gi in range(ogrp + (1 if ogrp_rem else 0)):
                    prep_group(gi, PREP_GRP if gi < ogrp else ogrp_rem, xtog, xbog, True)

            # ---------------- edge phase (per layer) ----------------
            def emit_layer(l, table):
                lo_ap = table.ap()
                hi_ap = table.ap()[lo_split:, :]
                with tc.tile_pool(name=f"g{l}", bufs=int(os.environ.get("KGB", "4"))) as gp, \
                     tc.tile_pool(name=f"e{l}", bufs=int(os.environ.get("KEB", "2"))) as ep, \
                     tc.tile_pool(name=f"s{l}", bufs=3) as sp, \
                     tc.tile_pool(name=f"scr{l}", bufs=6) as scrp, \
                     tc.tile_pool(name=f"oh{l}", bufs=6) as ohp, \
                     tc.tile_pool(name=f"fin{l}", bufs=2) as fp, \
                     tc.tile_pool(name=f"psT{l}", bufs=3, space="PSUM") as psT, \
                     tc.tile_pool(name=f"psW{l}", bufs=2, space="PSUM") as psW:
                    def finalize(w):
                        rows = 128 if w < nwin - 1 else last_win_rows
                        if l == 0:
                            ext1 = fp.tile([128, B_SLOT + 1], F32, tag="ext1")
                            h1b = ext1[:, 0:hh].bitcast(BF16)
                            nc.vector.tensor_copy(h1b, acc[:, w, :])
                            scr = scrp.tile([128, hidden], BF16, tag="escr")
                            nc.vector.scalar_tensor_tensor(
                                out=scr[:], in0=h1b, scalar=1.0, in1=gw_sb[:, 3, :],
                                op0=mybir.AluOpType.mult, op1=mybir.AluOpType.mult,
                                accum_out=ext1[:, B_SLOT:B_SLOT + 1])
                            scr2 = scrp.tile([128, hidden], BF16, tag="escr2")
                            a_f = scrp.tile([128, 1], F32, tag="af1")
                            nc.vector.scalar_tensor_tensor(
                                out=scr2[:], in0=h1b, scalar=1.0, in1=gw_sb[:, 2, :],
                                op0=mybir.AluOpType.mult, op1=mybir.AluOpType.mult,
                                accum_out=a_f[:])
                            nc.vector.tensor_scalar(out=a_arr[:, w, 1:2], in0=a_f[:],
                                                    scalar1=gb_sb[:, 1:2], scalar2=None,
                                                    op0=mybir.AluOpType.add)
                            nc.sync.dma_start(agi.ap()[w * 128:w * 128 + rows, 0:B_SLOT + 1],
                                              ext1[0:rows, :])

                    # windows with no lo-burst: seed acc with rawsc; fully
                    # edgeless windows also finalize immediately
                    for w in range(nwin):
                        if (0, w) not in burst:
                            nc.vector.tensor_copy(acc[:, w, :], rawsc[:, w, :])
                            if (1, w) not in burst:
                                finalize(w)

                    qi = 0
                    W_ps = None
                    for (h, ts, nt) in [c for c in calls]:
                        ct = nt // 128
                        t0 = ts // 128
                        G = gp.tile([128, CT_MAX, EXT_SLOTS], F32, tag="G")
                        nc.gpsimd.dma_gather(
                            out_ap=G[:, 0:ct, :],
                            in_ap=(hi_ap if h else lo_ap),
                            idxs_ap=idxt[:, ts // 16:(ts + nt) // 16],
                            num_idxs=nt, num_idxs_reg=nt, elem_size=EXT_SLOTS,
                            single_packet=False, queue_num=qi % 4)
                        qi += 1
                        ehT = ep.tile([128, CT_MAX * 128], BF16, tag="ehT")
                        nc.sync.dma_start(
                            ehT[:, 0:ct * 128],
                            bass.AP(eht_in, t0 * 128, [[e_tok, 128], [1, ct * 128]]))
                        atokP = psT.tile([128, CT_MAX], F32, tag="atokP")
                        # split call into window-pure runs
                        runs = []
                        c = 0
                        while c < ct:
                            w = tiles_w[t0 + c]
                            c1 = c
                            while c1 < ct and tiles_w[t0 + c1] == w:
                                c1 += 1
                            runs.append((w, c, c1))
                            c = c1
                        # pass 1: gate argument for the whole call
                        for (w, c0, c1) in runs:
                            for c in range(c0, c1):
                                nc.tensor.matmul(atokP[:, c:c + 1],
                                                 ehT[:, c * 128:(c + 1) * 128],
                                                 a_arr[:, w, l:l + 1],
                                                 start=True, stop=True)
                        arg = sp.tile([128, CT_MAX], F32, tag="arg")
                        nc.vector.tensor_tensor(out=arg[:, 0:ct], in0=atokP[:, 0:ct],
                                                in1=G[:, 0:ct, B_SLOT],
                                                op=mybir.AluOpType.add)
                        gt = sp.tile([128, CT_MAX], F32, tag="gt")
                        nc.scalar.activation(gt[:, 0:ct], arg[:, 0:ct],
                                             mybir.ActivationFunctionType.Tanh)
                        wt = sp.tile([128, CT_MAX], F32, tag="wt")
                        nc.vector.tensor_tensor(out=wt[:, 0:ct], in0=gt[:, 0:ct],
                                                in1=es_sb[:, t0:t0 + ct],
                                                op=mybir.AluOpType.mult)
                        # pass 2: one-hot scatter matmuls
                        for (w, c0, c1) in runs:
                            bf, bl = burst[(h, w)]
                            if t0 + c0 == bf:
                                W_ps = psW.tile([128, hidden], F32, tag="W")
                            for c in range(c0, c1):
                                oh = ohp.tile([128, 128], BF16, tag="oh")
                                nc.vector.tensor_scalar(
                                    out=oh[:], in0=iota_sb[:],
                                    scalar1=rr_sb[:, t0 + c:t0 + c + 1],
                                    scalar2=wt[:, c:c + 1],
                                    op0=mybir.AluOpType.is_equal,
                                    op1=mybir.AluOpType.mult)
                                nc.tensor.matmul(W_ps[:], oh[:], G[:, c, 0:hh].bitcast(BF16),
                                                 start=(t0 + c == bf),
                                                 stop=(t0 + c == bl))
                            if t0 + c1 - 1 == bl:
                                if h == 0:
                                    nc.vector.tensor_tensor(out=acc[:, w, :], in0=W_ps[:],
                                                            in1=rawsc[:, w, :],
                                                            op=mybir.AluOpType.add)
                                    if last_stream[w] == 0:
                                        finalize(w)
                                else:
                                    nc.vector.tensor_tensor(out=acc[:, w, :], in0=W_ps[:],
                                                            in1=acc[:, w, :],
                                                            op=mybir.AluOpType.add)
                                    finalize(w)

            phase = os.environ.get("KPHASE", "head")
            plvl = {"prep": 0, "l0": 1, "cc": 2, "l1": 3, "head": 4}[phase]
            if plvl >= 1:
                emit_layer(0, ext0)
            if plvl >= 2:
                nc.gpsimd.collective_compute(
                    "AllGather", mybir.AluOpType.bypass,
                    replica_groups=[list(range(ncores))],
                    ins=[agi.ap().opt()], outs=[ago.ap().opt()])
            if plvl >= 3:
                emit_layer(1, ago)
            if plvl < 4:
                with tc.tile_pool(name="zout", bufs=1) as zp:
                    o_z = zp.tile([128, out_ch], F32, tag="oz")
                    nc.vector.memset(o_z[:], 0.0)
                    for w in range(nwin):
                        rows = 128 if w < nwin - 1 else last_win_rows
                        nc.sync.dma_start(out.ap()[w * 128:w * 128 + rows, :],
                                          o_z[0:rows, :])
                return nc

            # ---------------- head: out = log_softmax(h @ t2^T + b) ----------
            # two passes so the Act engine loads the Exp/Ln tables once each
            with tc.tile_pool(name="head", bufs=4) as hp, \
                 tc.tile_pool(name="head_ps", bufs=4, space="PSUM") as hps:
                o_all = cp.tile([128, nwin, out_ch], F32, tag="o_all")
                nm_all = cp.tile([128, nwin], F32, tag="nm_all")
                s_all = cp.tile([128, nwin], F32, tag="s_all")
                for w in range(nwin):
                    ht_ps = hps.tile([128, 128], F32, tag="ht")
                    nc.tensor.matmul(ht_ps[:], acc[:, w, :], ident_sb[:],
                                     start=True, stop=True)
                    ht_sb = hp.tile([128, 128], F32, tag="ht_sb")
                    nc.vector.tensor_copy(ht_sb[:], ht_ps[:])
                    o_ps = hps.tile([128, out_ch], F32, tag="ops")
                    nc.tensor.matmul(o_ps[:], ht_sb[:], t2wt_sb[:], start=True, stop=False)
                    nc.tensor.matmul(o_ps[:], ones_sb[:], t2b_sb[:], start=False, stop=True)
                    nc.vector.reduce_max(out=nm_all[:, w:w + 1], in_=o_ps[:],
                                         axis=mybir.AxisListType.X, negate=True)
                    e_sb = hp.tile([128, out_ch], F32, tag="e")
                    nc.scalar.activation(e_sb[:], o_ps[:],
                                         mybir.ActivationFunctionType.Exp,
                                         bias=nm_all[:, w:w + 1])
                    nc.vector.reduce_sum(out=s_all[:, w:w + 1], in_=e_sb[:],
                                         axis=mybir.AxisListType.X)
                    nc.vector.tensor_copy(o_all[:, w, :], o_ps[:])
                ls_all = cp.tile([128, nwin], F32, tag="ls_all")
                nc.scalar.activation(ls_all[:], s_all[:], mybir.ActivationFunctionType.Ln)
                for w in range(nwin):
                    rows = 128 if w < nwin - 1 else last_win_rows
                    o_sb = hp.tile([128, out_ch], F32, tag="o")
                    nc.vector.tensor_scalar(out=o_sb[:], in0=o_all[:, w, :],
                                            scalar1=nm_all[:, w:w + 1],
                                            scalar2=ls_all[:, w:w + 1],
                                            op0=mybir.AluOpType.add,
                                            op1=mybir.AluOpType.subtract)
                    nc.sync.dma_start(out.ap()[w * 128:w * 128 + rows, :], o_sb[0:rows, :])

    return nc


# ======================================================================
# Host driver
# ======================================================================

def _bf16(a):
    import ml_dtypes
    return np.asarray(a, dtype=ml_dtypes.bfloat16)


def _group_x(xT_pad, nrow_units, kt):
    # xT_pad: [in_ch+1, units*128] f32 -> xtg [ngrp, 128, PREP_GRP*kt, 128],
    # xbg [ngrp, 1, PREP_GRP, 128] (ones row)
    in_ch = (xT_pad.shape[0] - 1)
    ngrp_t = (nrow_units + PREP_GRP - 1) // PREP_GRP
    pad_units = ngrp_t * PREP_GRP
    xp = np.zeros((in_ch + 1, pad_units * 128), np.float32)
    xp[:, :xT_pad.shape[1]] = xT_pad
    # [in, u, 128] -> [u, in, 128]
    xr = xp[:in_ch].reshape(in_ch, pad_units, 128).transpose(1, 0, 2)
    # [g, c, k, p, r] with in = k*128+p
    xg = xr.reshape(ngrp_t, PREP_GRP, kt, 128, 128)
    xtg = np.ascontiguousarray(xg.transpose(0, 3, 1, 2, 4)).reshape(
        ngrp_t, 128, PREP_GRP * kt, 128)
    xb = xp[in_ch].reshape(ngrp_t, 1, PREP_GRP, 128)
    return _bf16(xtg), _bf16(np.ascontiguousarray(xb))


def kernel_run(x, edge_index, t1_w, t1_b, gate_w, gate_b, t2_w, t2_b,
               n_nodes=N_NODES, in_ch=IN_CH, hidden=HIDDEN, out_ch=OUT_CH,
               eps=EPS, ncores=NCORES, lo_split=None, trace=False):
    _install_profile_hook()
    from concourse import bass_utils

    if lo_split is None:
        lo_split = min(25000, ((n_nodes + 1) // 2 + 127) // 128 * 128)
    meta = preprocess(edge_index, n_nodes, ncores, lo_split)
    nwin = meta["nwin"]
    r_per = n_nodes // ncores
    nchunk_tot = (n_nodes + 127) // 128
    kt = in_ch // 128

    nc = build_kernel(meta, n_nodes, in_ch, hidden, out_ch, eps, lo_split, ncores)
    nc.finalize()

    # host arrays
    x = np.asarray(x, np.float32)
    xT = np.concatenate([x.T, np.ones((1, x.shape[0]), np.float32)], axis=0)  # [in+1, N]
    pad_n = nchunk_tot * 128
    xT_pad = np.zeros((in_ch + 1, pad_n), np.float32)
    xT_pad[:, :n_nodes] = xT
    xtg_h, xbg_h = _group_x(xT_pad, nchunk_tot, kt)

    t1wt_h = _bf16(np.concatenate([np.asarray(t1_w, np.float32).T,
                                   np.asarray(t1_b, np.float32)[None, :]], axis=0))
    gw = np.asarray(gate_w, np.float32)
    gwrep_h = _bf16(np.stack([
        np.tile(gw[0, :hidden][None, :], (128, 1)),
        np.tile(gw[0, hidden:][None, :], (128, 1)),
        np.tile(gw[1, :hidden][None, :], (128, 1)),
        np.tile(gw[1, hidden:][None, :], (128, 1))]))
    gbrep_h = np.tile(np.asarray(gate_b, np.float32)[None, :], (128, 1))
    t2wt_h = np.ascontiguousarray(np.asarray(t2_w, np.float32).T)
    t2b_h = np.asarray(t2_b, np.float32)[None, :]
    iota_h = _bf16(np.tile(np.arange(128, dtype=np.float32)[None, :], (128, 1)))
    ident_h = np.eye(128, dtype=np.float32)
    ones_h = np.ones((1, 128), np.float32)

    in_maps = []
    for c in range(ncores):
        sl = np.zeros((in_ch + 1, nwin * 128), np.float32)
        take = min(nwin * 128, xT.shape[1] - c * r_per)
        sl[:, :take] = xT[:, c * r_per: c * r_per + take]
        xtog_h, xbog_h = _group_x(sl, nwin, kt)
        in_maps.append({
            "xtg": xtg_h, "xbg": xbg_h, "xtog": xtog_h, "xbog": xbog_h,
            "t1wt": t1wt_h, "gwrep": gwrep_h, "gbrep": gbrep_h,
            "t2wt": t2wt_h, "t2b": t2b_h,
            "iota": iota_h, "ident": ident_h, "ones": ones_h,
            "idx": meta["idx_dev"][c], "rr": meta["rr_dev"][c],
            "es": meta["es_dev"][c], "eht": meta["eht_dev"][c],
        })

    res = bass_utils.run_bass_kernel_spmd(
        nc, in_maps, core_ids=list(range(ncores)), trace=trace)
    outp = np.concatenate([res.results[c]["out"] for c in range(ncores)], axis=0)
    return outp[:n_nodes], res


def kernel(**inputs):
    x = inputs["x"]
    edge_index = inputs["edge_index"]
    outp, _ = kernel_run(
        x, edge_index, inputs["t1_w"], inputs["t1_b"], inputs["gate_w"],
        inputs["gate_b"], inputs["t2_w"], inputs["t2_b"])
    return np.asarray(outp, np.float32)



# revision 9
# speedup vs baseline: 1.9717x; 1.9717x over previous
"""FAGCN forward on 8 TRN2 NeuronCores (Bass/Tile).

Sharding: row-partition of nodes, 8 ways. The dense input projection
(h = relu(x @ t1^T + b)) is replicated on every core into a 512B-stride
gather table [h bf16 x128 | b f32 | pad] (prep batched 8 chunks/group).
Per layer the edge phase is a two-stream token walk ([all-lo windows]
[all-hi windows], int16 gather indices split at lo_split): 1024-edge
dma_gather calls (small calls + 4 G buffers let the Q7 desc-gen
pipeline two calls at once — the binding resource) fetch source rows.
The gate's a[row] term is recovered on TensorE: a host-precomputed
transposed one-hot (eht, streamed via plain DMA) is the stationary of a
[tok,1] matmul against the per-window a vector; one tanh per call turns
a+b into edge weights. The scatter-add is a per-tile fused
(iota==rr)*w one-hot (single DVE op) feeding a TensorE matmul into a
per-window PSUM accumulator; windows accumulate in SBUF across the two
streams. Between layers the owned rows are AllGathered. The head
(t2 matmul + log_softmax) runs as a final two-pass sweep so the Exp/Ln
activation tables load once each.
"""

import os
import sys
import numpy as np

sys.path.insert(0, "/opt/trn_rl_repo")

import concourse.bass as bass
import concourse.bacc as bacc
import concourse.mybir as mybir
import concourse.tile as tile
from concourse import library_config

F32 = mybir.dt.float32
BF16 = mybir.dt.bfloat16
I16 = mybir.dt.int16

# problem constants (self-contained per contract)
N_NODES = 50000
IN_CH = 256
HIDDEN = 128
OUT_CH = 64
EPS = 0.3
NCORES = 8
CALL_TOKENS = int(os.environ.get("KCT", "1024"))
CT_MAX = CALL_TOKENS // 128
EXT_SLOTS = 128   # 512B gather record
B_SLOT = 64       # f32 slot holding the gate b-term
PREP_GRP = 8


def _install_profile_hook():
    import types
    name = "antenv.axon_hooks"
    if name in sys.modules:
        return
    try:
        import trn_agent_boot.trn_boot as tb
        hook = tb._ntff_profile_via_ctypes("/opt/axon/libaxon_pjrt.so")
    except Exception:
        hook = None
    mod = types.ModuleType(name)
    mod._hook = hook
    mod.get_axon_ntff_profile_hook = lambda: mod._hook
    mod.set_axon_ntff_profile_hook = lambda h: setattr(mod, "_hook", h)
    sys.modules[name] = mod


# ======================================================================
# Host preprocessing: SPMD token streams + per-core data
# ======================================================================

def preprocess(edge_index, n_nodes, ncores, lo_split):
    row = np.asarray(edge_index[0], dtype=np.int64)
    col = np.asarray(edge_index[1], dtype=np.int64)
    E = row.shape[0]
    r_per = n_nodes // ncores
    nwin = (r_per + 127) // 128

    deg = np.bincount(row, minlength=n_nodes).astype(np.float64)
    dinv = np.where(deg > 0, 1.0 / np.sqrt(np.maximum(deg, 1.0)), 0.0)
    escale_all = (dinv[row] * dinv[col]).astype(np.float32)

    core = row // r_per
    lrow = row - core * r_per
    win = lrow // 128
    is_hi = (col >= lo_split).astype(np.int64)

    # stream order: core, then stream (lo/hi), then window, then lrow
    order = np.lexsort((lrow, win, is_hi, core))
    core_s, win_s, hi_s = core[order], win[order], is_hi[order]
    lrow_s, col_s, esc_s = lrow[order], col[order], escale_all[order]

    key = (core_s * 2 + hi_s) * nwin + win_s
    cnt = np.bincount(key, minlength=ncores * 2 * nwin).reshape(ncores, 2, nwin)
    sec_len = ((cnt.max(axis=0) + 127) // 128) * 128  # [2, nwin]
    L_lo = int(sec_len[0].sum())
    L_hi = int(sec_len[1].sum())
    e_tok = L_lo + L_hi
    sec_start = np.zeros((2, nwin), np.int64)
    sec_start[0] = np.concatenate([[0], np.cumsum(sec_len[0])[:-1]])
    sec_start[1] = L_lo + np.concatenate([[0], np.cumsum(sec_len[1])[:-1]])

    col16 = np.zeros((ncores, e_tok), np.int16)
    rowrel = np.zeros((ncores, e_tok), np.float32)
    esc = np.zeros((ncores, e_tok), np.float32)

    grp_first = np.zeros(ncores * 2 * nwin + 1, np.int64)
    np.cumsum(cnt.reshape(-1), out=grp_first[1:])
    rank = np.arange(E) - grp_first[key]
    dest = sec_start[hi_s, win_s] + rank
    cval = np.where(hi_s == 1, col_s - lo_split, col_s).astype(np.int16)
    col16[core_s, dest] = cval
    rowrel[core_s, dest] = (lrow_s - win_s * 128).astype(np.float32)
    esc[core_s, dest] = esc_s

    # gather calls per stream
    calls = []  # (stream, ts, nt)
    for h, base, L in ((0, 0, L_lo), (1, L_lo, L_hi)):
        off = 0
        while off < L:
            nt = min(CALL_TOKENS, L - off)
            calls.append((h, base + off, nt))
            off += nt

    idx_dev = np.zeros((ncores, 128, e_tok // 16), np.int16)
    for (h, ts, nt) in calls:
        blk = col16[:, ts:ts + nt].reshape(ncores, nt // 16, 16)
        blk = np.ascontiguousarray(np.transpose(blk, (0, 2, 1)))
        idx_dev[:, :, ts // 16:(ts + nt) // 16] = np.tile(blk, (1, 8, 1))
    rr_dev = np.ascontiguousarray(rowrel.reshape(ncores, -1, 128).transpose(0, 2, 1))
    es_dev = np.ascontiguousarray(esc.reshape(ncores, -1, 128).transpose(0, 2, 1))

    # transposed one-hot, partition-major: eht[r, i] = 1 iff rowrel[i] == r
    import ml_dtypes
    eht = np.zeros((ncores, 128 * e_tok), ml_dtypes.bfloat16)
    ii = np.arange(e_tok)
    for c in range(ncores):
        eht[c][rowrel[c].astype(np.int64) * e_tok + ii] = 1.0
    eht = eht.reshape(ncores, 128, e_tok)

    return {
        "nwin": nwin, "e_tok": e_tok, "sec_len": sec_len, "calls": calls,
        "idx_dev": idx_dev, "rr_dev": rr_dev, "es_dev": es_dev, "eht_dev": eht,
    }


# ======================================================================
# Kernel builder
# ======================================================================

def build_kernel(meta, n_nodes, in_ch, hidden, out_ch, eps, lo_split, ncores):
    nwin = meta["nwin"]
    e_tok = meta["e_tok"]
    sec_len = meta["sec_len"]
    calls = meta["calls"]
    nchunk_tot = (n_nodes + 127) // 128
    r_per = n_nodes // ncores
    last_win_rows = r_per - 128 * (nwin - 1)
    kt = in_ch // 128
    hh = hidden // 2  # f32 slots holding the bf16 h vector

    # tile -> window map, and burst boundaries per (stream, window)
    tiles_w = []
    burst = {}  # (h, w) -> (gfirst, glast) in global tile idx
    for h in range(2):
        for w in range(nwin):
            ntl = int(sec_len[h, w]) // 128
            if ntl == 0:
                continue
            g0 = len(tiles_w)
            tiles_w.extend([w] * ntl)
            burst[(h, w)] = (g0, g0 + ntl - 1)
    assert len(tiles_w) == e_tok // 128
    last_stream = {}
    for w in range(nwin):
        last_stream[w] = 1 if (1, w) in burst else 0

    ngrp = nchunk_tot // PREP_GRP
    grp_rem = nchunk_tot - ngrp * PREP_GRP
    ogrp = nwin // PREP_GRP
    ogrp_rem = nwin - ogrp * PREP_GRP

    nc = bacc.Bacc("TRN2", target_bir_lowering=False, debug=False,
                   num_devices=ncores, num_swdge_queues=4)

    # ---- I/O ----
    # xtg: host-prearranged [group, 128p, grp*kt, 128] bf16 (+ ones row separately)
    xtg = nc.dram_tensor("xtg", [ngrp + (1 if grp_rem else 0), 128, PREP_GRP * kt, 128], BF16, kind="ExternalInput")
    xbg = nc.dram_tensor("xbg", [ngrp + (1 if grp_rem else 0), 1, PREP_GRP, 128], BF16, kind="ExternalInput")
    xtog = nc.dram_tensor("xtog", [ogrp + (1 if ogrp_rem else 0), 128, PREP_GRP * kt, 128], BF16, kind="ExternalInput")
    xbog = nc.dram_tensor("xbog", [ogrp + (1 if ogrp_rem else 0), 1, PREP_GRP, 128], BF16, kind="ExternalInput")
    t1wt = nc.dram_tensor("t1wt", [in_ch + 1, hidden], BF16, kind="ExternalInput")
    gwrep = nc.dram_tensor("gwrep", [4, 128, hidden], BF16, kind="ExternalInput")
    gbrep = nc.dram_tensor("gbrep", [128, 2], F32, kind="ExternalInput")
    t2wt = nc.dram_tensor("t2wt", [hidden, out_ch], F32, kind="ExternalInput")
    t2b = nc.dram_tensor("t2b", [1, out_ch], F32, kind="ExternalInput")
    iota_in = nc.dram_tensor("iota", [128, 128], BF16, kind="ExternalInput")
    iotac_in = nc.dram_tensor("iotac", [128, CT_MAX * 128], BF16, kind="ExternalInput")
    ident_in = nc.dram_tensor("ident", [128, 128], F32, kind="ExternalInput")
    ones_in = nc.dram_tensor("ones", [1, 128], F32, kind="ExternalInput")
    idx_in = nc.dram_tensor("idx", [128, e_tok // 16], I16, kind="ExternalInput")
    eht_in = nc.dram_tensor("eht", [128, e_tok], BF16, kind="ExternalInput")
    rr_in = nc.dram_tensor("rr", [128, e_tok // 128], F32, kind="ExternalInput")
    rrb_in = nc.dram_tensor("rrb", [128, e_tok // 128], BF16, kind="ExternalInput")
    es_in = nc.dram_tensor("es", [128, e_tok // 128], F32, kind="ExternalInput")
    out = nc.dram_tensor("out", [r_per, out_ch], F32, kind="ExternalOutput")

    ext0 = nc.dram_tensor("ext0", [nchunk_tot * 128, EXT_SLOTS], F32)
    agi = nc.dram_tensor("agi", [r_per, EXT_SLOTS], F32)
    ago = nc.dram_tensor("ago", [r_per * ncores, EXT_SLOTS], F32)

    with tile.TileContext(nc) as tc:
        nc.gpsimd.load_library(library_config.mlp)
        with tc.tile_pool(name="consts", bufs=1) as cp:
            t1wt_sb = cp.tile([128, kt, hidden], BF16, tag="t1wt")
            nc.sync.dma_start(t1wt_sb[:], bass.AP(t1wt, 0, [[hidden, 128], [128 * hidden, kt], [1, hidden]]))
            t1b_sb = cp.tile([1, hidden], BF16, tag="t1b")
            nc.sync.dma_start(t1b_sb[:], t1wt.ap()[in_ch:in_ch + 1, :])
            gw_sb = cp.tile([128, 4, hidden], BF16, tag="gw")
            nc.sync.dma_start(gw_sb[:], bass.AP(gwrep, 0, [[hidden, 128], [128 * hidden, 4], [1, hidden]]))
            gb_sb = cp.tile([128, 2], F32, tag="gb")
            nc.sync.dma_start(gb_sb[:], gbrep.ap())
            t2wt_sb = cp.tile([128, out_ch], F32, tag="t2wt")
            nc.sync.dma_start(t2wt_sb[:], t2wt.ap())
            t2b_sb = cp.tile([1, out_ch], F32, tag="t2b")
            nc.sync.dma_start(t2b_sb[:], t2b.ap())
            iota_sb = cp.tile([128, 128], BF16, tag="iota")
            nc.sync.dma_start(iota_sb[:], iota_in.ap())
            ident_sb = cp.tile([128, 128], F32, tag="ident")
            nc.sync.dma_start(ident_sb[:], ident_in.ap())
            ones_sb = cp.tile([1, 128], F32, tag="ones")
            nc.sync.dma_start(ones_sb[:], ones_in.ap())
            idxt = cp.tile([128, e_tok // 16], I16, tag="idxt")
            nc.sync.dma_start(idxt[:], idx_in.ap())
            rr_sb = cp.tile([128, e_tok // 128], F32, tag="rr")
            nc.sync.dma_start(rr_sb[:], rr_in.ap())
            rrb_sb = cp.tile([128, e_tok // 128], BF16, tag="rrb")
            nc.sync.dma_start(rrb_sb[:], rrb_in.ap())
            es_sb = cp.tile([128, e_tok // 128], F32, tag="es")
            nc.sync.dma_start(es_sb[:], es_in.ap())
            iotac_sb = cp.tile([128, CT_MAX, 128], BF16, tag="iotac")
            nc.sync.dma_start(iotac_sb[:], iotac_in.ap())

            rawsc = cp.tile([128, nwin, hidden], F32, tag="rawsc")
            acc = cp.tile([128, nwin, hidden], F32, tag="acc")
            a_arr = cp.tile([128, nwin, 2], BF16, tag="a_arr")

            # ---------------- prep: replicated gather table ----------------
            with tc.tile_pool(name="prep", bufs=3) as pp, \
                 tc.tile_pool(name="prep_s", bufs=6) as pscr, \
                 tc.tile_pool(name="prep_ps", bufs=4, space="PSUM") as pps:

                def prep_group(gi, gcnt, xt_t, xb_t, own):
                    xt_sb = pp.tile([128, PREP_GRP * kt, 128], BF16, tag="xt")
                    nc.sync.dma_start(xt_sb[:, 0:gcnt * kt, :], xt_t.ap()[gi, :, 0:gcnt * kt, :])
                    xb_sb = pp.tile([1, PREP_GRP, 128], BF16, tag="xb")
                    nc.sync.dma_start(xb_sb[:, 0:gcnt, :], xb_t.ap()[gi, :, 0:gcnt, :])
                    if not own:
                        extg = pp.tile([128, PREP_GRP, B_SLOT + 1], F32, tag="extg")
                    for c in range(gcnt):
                        ps = pps.tile([128, hidden], F32, tag="h0ps")
                        for k in range(kt):
                            nc.tensor.matmul(ps[:], xt_sb[:, c * kt + k, :], t1wt_sb[:, k, :],
                                             start=(k == 0), stop=False)
                        nc.tensor.matmul(ps[:], xb_sb[:, c, :], t1b_sb[:], start=False, stop=True)
                        if own:
                            w = gi * PREP_GRP + c
                            nc.vector.tensor_scalar(out=rawsc[:, w, :], in0=ps[:],
                                                    scalar1=0.0, scalar2=eps,
                                                    op0=mybir.AluOpType.max,
                                                    op1=mybir.AluOpType.mult)
                            hb = pscr.tile([128, hidden], BF16, tag="hb")
                            nc.scalar.activation(hb[:], ps[:], mybir.ActivationFunctionType.Relu)
                            scr = pscr.tile([128, hidden], BF16, tag="scr")
                            a_f = pscr.tile([128, 1], F32, tag="af")
                            nc.vector.scalar_tensor_tensor(
                                out=scr[:], in0=hb[:], scalar=1.0, in1=gw_sb[:, 0, :],
                                op0=mybir.AluOpType.mult, op1=mybir.AluOpType.mult,
                                accum_out=a_f[:])
                            nc.vector.tensor_scalar(out=a_arr[:, w, 0:1], in0=a_f[:],
                                                    scalar1=gb_sb[:, 0:1], scalar2=None,
                                                    op0=mybir.AluOpType.add)
                        else:
                            hb = extg[:, c, 0:hh].bitcast(BF16)
                            nc.scalar.activation(hb, ps[:], mybir.ActivationFunctionType.Relu)
                            scr = pscr.tile([128, hidden], BF16, tag="scr")
                            nc.vector.scalar_tensor_tensor(
                                out=scr[:], in0=hb, scalar=1.0, in1=gw_sb[:, 1, :],
                                op0=mybir.AluOpType.mult, op1=mybir.AluOpType.mult,
                                accum_out=extg[:, c, B_SLOT:B_SLOT + 1])
                    if not own:
                        base = gi * PREP_GRP * 128
                        nc.sync.dma_start(
                            bass.AP(ext0, base * EXT_SLOTS,
                                    [[EXT_SLOTS, 128], [128 * EXT_SLOTS, gcnt], [1, B_SLOT + 1]]),
                            extg[:, 0:gcnt, :])

                for gi in range(ngrp + (1 if grp_rem else 0)):
                    prep_group(gi, PREP_GRP if gi < ngrp else grp_rem, xtg, xbg, False)
                for gi in range(ogrp + (1 if ogrp_rem else 0)):
                    prep_group(gi, PREP_GRP if gi < ogrp else ogrp_rem, xtog, xbog, True)

            # ---------------- edge phase (per layer) ----------------
            def emit_layer(l, table):
                lo_ap = table.ap()
                hi_ap = table.ap()[lo_split:, :]
                with tc.tile_pool(name=f"g{l}", bufs=int(os.environ.get("KGB", "4"))) as gp, \
                     tc.tile_pool(name=f"e{l}", bufs=int(os.environ.get("KEB", "2"))) as ep, \
                     tc.tile_pool(name=f"s{l}", bufs=3) as sp, \
                     tc.tile_pool(name=f"scr{l}", bufs=6) as scrp, \
                     tc.tile_pool(name=f"oh{l}", bufs=3) as ohp, \
                     tc.tile_pool(name=f"fin{l}", bufs=2) as fp, \
                     tc.tile_pool(name=f"psT{l}", bufs=3, space="PSUM") as psT, \
                     tc.tile_pool(name=f"psW{l}", bufs=2, space="PSUM") as psW:
                    def finalize(w):
                        rows = 128 if w < nwin - 1 else last_win_rows
                        if l == 0:
                            ext1 = fp.tile([128, B_SLOT + 1], F32, tag="ext1")
                            h1b = ext1[:, 0:hh].bitcast(BF16)
                            nc.vector.tensor_copy(h1b, acc[:, w, :])
                            scr = scrp.tile([128, hidden], BF16, tag="escr")
                            nc.vector.scalar_tensor_tensor(
                                out=scr[:], in0=h1b, scalar=1.0, in1=gw_sb[:, 3, :],
                                op0=mybir.AluOpType.mult, op1=mybir.AluOpType.mult,
                                accum_out=ext1[:, B_SLOT:B_SLOT + 1])
                            scr2 = scrp.tile([128, hidden], BF16, tag="escr2")
                            a_f = scrp.tile([128, 1], F32, tag="af1")
                            nc.vector.scalar_tensor_tensor(
                                out=scr2[:], in0=h1b, scalar=1.0, in1=gw_sb[:, 2, :],
                                op0=mybir.AluOpType.mult, op1=mybir.AluOpType.mult,
                                accum_out=a_f[:])
                            nc.vector.tensor_scalar(out=a_arr[:, w, 1:2], in0=a_f[:],
                                                    scalar1=gb_sb[:, 1:2], scalar2=None,
                                                    op0=mybir.AluOpType.add)
                            nc.sync.dma_start(agi.ap()[w * 128:w * 128 + rows, 0:B_SLOT + 1],
                                              ext1[0:rows, :])

                    # windows with no lo-burst: seed acc with rawsc; fully
                    # edgeless windows also finalize immediately
                    for w in range(nwin):
                        if (0, w) not in burst:
                            nc.vector.tensor_copy(acc[:, w, :], rawsc[:, w, :])
                            if (1, w) not in burst:
                                finalize(w)

                    qi = 0
                    W_ps = None
                    for (h, ts, nt) in [c for c in calls]:
                        ct = nt // 128
                        t0 = ts // 128
                        G = gp.tile([128, CT_MAX, EXT_SLOTS], F32, tag="G")
                        nc.gpsimd.dma_gather(
                            out_ap=G[:, 0:ct, :],
                            in_ap=(hi_ap if h else lo_ap),
                            idxs_ap=idxt[:, ts // 16:(ts + nt) // 16],
                            num_idxs=nt, num_idxs_reg=nt, elem_size=EXT_SLOTS,
                            single_packet=False, queue_num=qi % 4)
                        qi += 1
                        ehT = ep.tile([128, CT_MAX * 128], BF16, tag="ehT")
                        nc.sync.dma_start(
                            ehT[:, 0:ct * 128],
                            bass.AP(eht_in, t0 * 128, [[e_tok, 128], [1, ct * 128]]))
                        atokP = psT.tile([128, CT_MAX], F32, tag="atokP")
                        # split call into window-pure runs
                        runs = []
                        c = 0
                        while c < ct:
                            w = tiles_w[t0 + c]
                            c1 = c
                            while c1 < ct and tiles_w[t0 + c1] == w:
                                c1 += 1
                            runs.append((w, c, c1))
                            c = c1
                        # pass 1: gate argument for the whole call
                        for (w, c0, c1) in runs:
                            for c in range(c0, c1):
                                nc.tensor.matmul(atokP[:, c:c + 1],
                                                 ehT[:, c * 128:(c + 1) * 128],
                                                 a_arr[:, w, l:l + 1],
                                                 start=True, stop=True)
                        arg = sp.tile([128, CT_MAX], F32, tag="arg")
                        nc.vector.tensor_tensor(out=arg[:, 0:ct], in0=atokP[:, 0:ct],
                                                in1=G[:, 0:ct, B_SLOT],
                                                op=mybir.AluOpType.add)
                        gt = sp.tile([128, CT_MAX], BF16, tag="gt")
                        nc.scalar.activation(gt[:, 0:ct], arg[:, 0:ct],
                                             mybir.ActivationFunctionType.Tanh)
                        wt = sp.tile([128, CT_MAX], BF16, tag="wt")
                        nc.vector.tensor_tensor(out=wt[:, 0:ct], in0=gt[:, 0:ct],
                                                in1=es_sb[:, t0:t0 + ct],
                                                op=mybir.AluOpType.mult)
                        # batched one-hot: ohc[p, c, f] = (iota[f]==rr[p,c]) * wt[p,c]
                        eqc = ohp.tile([128, CT_MAX, 128], BF16, tag="eqc")
                        rrs = rrb_sb[:, t0:t0 + ct]
                        rrx = bass.AP(rrs.tensor, rrs.offset, list(rrs.ap) + [[0, 128]])
                        nc.vector.tensor_tensor(out=eqc[:, 0:ct, :],
                                                in0=iotac_sb[:, 0:ct, :], in1=rrx,
                                                op=mybir.AluOpType.is_equal)
                        ohc = ohp.tile([128, CT_MAX, 128], BF16, tag="ohc")
                        wts = wt[:, 0:ct]
                        wtx = bass.AP(wts.tensor, wts.offset, list(wts.ap) + [[0, 128]])
                        nc.vector.tensor_tensor(out=ohc[:, 0:ct, :],
                                                in0=eqc[:, 0:ct, :], in1=wtx,
                                                op=mybir.AluOpType.mult)
                        # pass 2: one-hot scatter matmuls
                        for (w, c0, c1) in runs:
                            bf, bl = burst[(h, w)]
                            if t0 + c0 == bf:
                                W_ps = psW.tile([128, hidden], F32, tag="W")
                            for c in range(c0, c1):
                                nc.tensor.matmul(W_ps[:], ohc[:, c, :], G[:, c, 0:hh].bitcast(BF16),
                                                 start=(t0 + c == bf),
                                                 stop=(t0 + c == bl))
                            if t0 + c1 - 1 == bl:
                                if h == 0:
                                    nc.vector.tensor_tensor(out=acc[:, w, :], in0=W_ps[:],
                                                            in1=rawsc[:, w, :],
                                                            op=mybir.AluOpType.add)
                                    if last_stream[w] == 0:
                                        finalize(w)
                                else:
                                    nc.vector.tensor_tensor(out=acc[:, w, :], in0=W_ps[:],
                                                            in1=acc[:, w, :],
                                                            op=mybir.AluOpType.add)
                                    finalize(w)

            phase = os.environ.get("KPHASE", "head")
            plvl = {"prep": 0, "l0": 1, "cc": 2, "l1": 3, "head": 4}[phase]
            if plvl >= 1:
                emit_layer(0, ext0)
            if plvl >= 2:
                nc.gpsimd.collective_compute(
                    "AllGather", mybir.AluOpType.bypass,
                    replica_groups=[list(range(ncores))],
                    ins=[agi.ap().opt()], outs=[ago.ap().opt()])
            if plvl >= 3:
                emit_layer(1, ago)
            if plvl < 4:
                with tc.tile_pool(name="zout", bufs=1) as zp:
                    o_z = zp.tile([128, out_ch], F32, tag="oz")
                    nc.vector.memset(o_z[:], 0.0)
                    for w in range(nwin):
                        rows = 128 if w < nwin - 1 else last_win_rows
                        nc.sync.dma_start(out.ap()[w * 128:w * 128 + rows, :],
                                          o_z[0:rows, :])
                return nc

            # ---------------- head: out = log_softmax(h @ t2^T + b) ----------
            # two passes so the Act engine loads the Exp/Ln tables once each
            with tc.tile_pool(name="head", bufs=4) as hp, \
                 tc.tile_pool(name="head_ps", bufs=4, space="PSUM") as hps:
                o_all = cp.tile([128, nwin, out_ch], F32, tag="o_all")
                nm_all = cp.tile([128, nwin], F32, tag="nm_all")
                s_all = cp.tile([128, nwin], F32, tag="s_all")
                for w in range(nwin):
                    ht_ps = hps.tile([128, 128], F32, tag="ht")
                    nc.tensor.matmul(ht_ps[:], acc[:, w, :], ident_sb[:],
                                     start=True, stop=True)
                    ht_sb = hp.tile([128, 128], F32, tag="ht_sb")
                    nc.vector.tensor_copy(ht_sb[:], ht_ps[:])
                    o_ps = hps.tile([128, out_ch], F32, tag="ops")
                    nc.tensor.matmul(o_ps[:], ht_sb[:], t2wt_sb[:], start=True, stop=False)
                    nc.tensor.matmul(o_ps[:], ones_sb[:], t2b_sb[:], start=False, stop=True)
                    nc.vector.reduce_max(out=nm_all[:, w:w + 1], in_=o_ps[:],
                                         axis=mybir.AxisListType.X, negate=True)
                    e_sb = hp.tile([128, out_ch], F32, tag="e")
                    nc.scalar.activation(e_sb[:], o_ps[:],
                                         mybir.ActivationFunctionType.Exp,
                                         bias=nm_all[:, w:w + 1])
                    nc.vector.reduce_sum(out=s_all[:, w:w + 1], in_=e_sb[:],
                                         axis=mybir.AxisListType.X)
                    nc.vector.tensor_copy(o_all[:, w, :], o_ps[:])
                ls_all = cp.tile([128, nwin], F32, tag="ls_all")
                nc.scalar.activation(ls_all[:], s_all[:], mybir.ActivationFunctionType.Ln)
                for w in range(nwin):
                    rows = 128 if w < nwin - 1 else last_win_rows
                    o_sb = hp.tile([128, out_ch], F32, tag="o")
                    nc.vector.tensor_scalar(out=o_sb[:], in0=o_all[:, w, :],
                                            scalar1=nm_all[:, w:w + 1],
                                            scalar2=ls_all[:, w:w + 1],
                                            op0=mybir.AluOpType.add,
                                            op1=mybir.AluOpType.subtract)
                    nc.sync.dma_start(out.ap()[w * 128:w * 128 + rows, :], o_sb[0:rows, :])

    return nc


# ======================================================================
# Host driver
# ======================================================================

def _bf16(a):
    import ml_dtypes
    return np.asarray(a, dtype=ml_dtypes.bfloat16)


def _group_x(xT_pad, nrow_units, kt):
    # xT_pad: [in_ch+1, units*128] f32 -> xtg [ngrp, 128, PREP_GRP*kt, 128],
    # xbg [ngrp, 1, PREP_GRP, 128] (ones row)
    in_ch = (xT_pad.shape[0] - 1)
    ngrp_t = (nrow_units + PREP_GRP - 1) // PREP_GRP
    pad_units = ngrp_t * PREP_GRP
    xp = np.zeros((in_ch + 1, pad_units * 128), np.float32)
    xp[:, :xT_pad.shape[1]] = xT_pad
    # [in, u, 128] -> [u, in, 128]
    xr = xp[:in_ch].reshape(in_ch, pad_units, 128).transpose(1, 0, 2)
    # [g, c, k, p, r] with in = k*128+p
    xg = xr.reshape(ngrp_t, PREP_GRP, kt, 128, 128)
    xtg = np.ascontiguousarray(xg.transpose(0, 3, 1, 2, 4)).reshape(
        ngrp_t, 128, PREP_GRP * kt, 128)
    xb = xp[in_ch].reshape(ngrp_t, 1, PREP_GRP, 128)
    return _bf16(xtg), _bf16(np.ascontiguousarray(xb))


def kernel_run(x, edge_index, t1_w, t1_b, gate_w, gate_b, t2_w, t2_b,
               n_nodes=N_NODES, in_ch=IN_CH, hidden=HIDDEN, out_ch=OUT_CH,
               eps=EPS, ncores=NCORES, lo_split=None, trace=False):
    _install_profile_hook()
    from concourse import bass_utils

    if lo_split is None:
        lo_split = min(25000, ((n_nodes + 1) // 2 + 127) // 128 * 128)
    meta = preprocess(edge_index, n_nodes, ncores, lo_split)
    nwin = meta["nwin"]
    r_per = n_nodes // ncores
    nchunk_tot = (n_nodes + 127) // 128
    kt = in_ch // 128

    nc = build_kernel(meta, n_nodes, in_ch, hidden, out_ch, eps, lo_split, ncores)
    nc.finalize()

    # host arrays
    x = np.asarray(x, np.float32)
    xT = np.concatenate([x.T, np.ones((1, x.shape[0]), np.float32)], axis=0)  # [in+1, N]
    pad_n = nchunk_tot * 128
    xT_pad = np.zeros((in_ch + 1, pad_n), np.float32)
    xT_pad[:, :n_nodes] = xT
    xtg_h, xbg_h = _group_x(xT_pad, nchunk_tot, kt)

    t1wt_h = _bf16(np.concatenate([np.asarray(t1_w, np.float32).T,
                                   np.asarray(t1_b, np.float32)[None, :]], axis=0))
    gw = np.asarray(gate_w, np.float32)
    gwrep_h = _bf16(np.stack([
        np.tile(gw[0, :hidden][None, :], (128, 1)),
        np.tile(gw[0, hidden:][None, :], (128, 1)),
        np.tile(gw[1, :hidden][None, :], (128, 1)),
        np.tile(gw[1, hidden:][None, :], (128, 1))]))
    gbrep_h = np.tile(np.asarray(gate_b, np.float32)[None, :], (128, 1))
    t2wt_h = np.ascontiguousarray(np.asarray(t2_w, np.float32).T)
    t2b_h = np.asarray(t2_b, np.float32)[None, :]
    iota_h = _bf16(np.tile(np.arange(128, dtype=np.float32)[None, :], (128, 1)))
    iotac_h = _bf16(np.tile(np.arange(128, dtype=np.float32)[None, :],
                            (128, CT_MAX)))
    ident_h = np.eye(128, dtype=np.float32)
    ones_h = np.ones((1, 128), np.float32)

    in_maps = []
    for c in range(ncores):
        sl = np.zeros((in_ch + 1, nwin * 128), np.float32)
        take = min(nwin * 128, xT.shape[1] - c * r_per)
        sl[:, :take] = xT[:, c * r_per: c * r_per + take]
        xtog_h, xbog_h = _group_x(sl, nwin, kt)
        in_maps.append({
            "xtg": xtg_h, "xbg": xbg_h, "xtog": xtog_h, "xbog": xbog_h,
            "t1wt": t1wt_h, "gwrep": gwrep_h, "gbrep": gbrep_h,
            "t2wt": t2wt_h, "t2b": t2b_h,
            "iota": iota_h, "iotac": iotac_h, "ident": ident_h, "ones": ones_h,
            "idx": meta["idx_dev"][c], "rr": meta["rr_dev"][c],
            "rrb": _bf16(meta["rr_dev"][c]),
            "es": meta["es_dev"][c], "eht": meta["eht_dev"][c],
        })

    res = bass_utils.run_bass_kernel_spmd(
        nc, in_maps, core_ids=list(range(ncores)), trace=trace)
    outp = np.concatenate([res.results[c]["out"] for c in range(ncores)], axis=0)
    return outp[:n_nodes], res


def kernel(**inputs):
    x = inputs["x"]
    edge_index = inputs["edge_index"]
    outp, _ = kernel_run(
        x, edge_index, inputs["t1_w"], inputs["t1_b"], inputs["gate_w"],
        inputs["gate_b"], inputs["t2_w"], inputs["t2_b"])
    return np.asarray(outp, np.float32)



# revision 17
# speedup vs baseline: 2.0166x; 1.0228x over previous
"""FAGCN forward on 8 TRN2 NeuronCores (Bass/Tile).

Sharding: row-partition of nodes, 8 ways. The dense input projection
(h = relu(x @ t1^T + b)) is replicated on every core into a 512B-stride
gather table [h bf16 x128 | b f32 | pad] (prep batched 8 chunks/group).
Per layer the edge phase is a two-stream token walk ([all-lo windows]
[all-hi windows], int16 gather indices split at lo_split): 1024-edge
dma_gather calls (small calls + 4 G buffers let the Q7 desc-gen
pipeline two calls at once — the binding resource) fetch source rows.
The gate's a[row] term is recovered on TensorE: a host-precomputed
transposed one-hot (eht, streamed via plain DMA) is the stationary of a
[tok,1] matmul against the per-window a vector; one tanh per call turns
a+b into edge weights. The scatter-add is a per-tile fused
(iota==rr)*w one-hot (single DVE op) feeding a TensorE matmul into a
per-window PSUM accumulator; windows accumulate in SBUF across the two
streams. Between layers the owned rows are AllGathered. The head
(t2 matmul + log_softmax) runs as a final two-pass sweep so the Exp/Ln
activation tables load once each.
"""

import os
import sys
import numpy as np

sys.path.insert(0, "/opt/trn_rl_repo")

import concourse.bass as bass
import concourse.bacc as bacc
import concourse.mybir as mybir
import concourse.tile as tile
from concourse import library_config

F32 = mybir.dt.float32
BF16 = mybir.dt.bfloat16
I16 = mybir.dt.int16

# problem constants (self-contained per contract)
N_NODES = 50000
IN_CH = 256
HIDDEN = 128
OUT_CH = 64
EPS = 0.3
NCORES = 8
CALL_TOKENS = int(os.environ.get("KCT", "1024"))
CT_MAX = CALL_TOKENS // 128
EXT_SLOTS = 128   # 512B gather record
B_SLOT = 64       # f32 slot holding the gate b-term
PREP_GRP = 8


def _install_profile_hook():
    import types
    name = "antenv.axon_hooks"
    if name in sys.modules:
        return
    try:
        import trn_agent_boot.trn_boot as tb
        hook = tb._ntff_profile_via_ctypes("/opt/axon/libaxon_pjrt.so")
    except Exception:
        hook = None
    mod = types.ModuleType(name)
    mod._hook = hook
    mod.get_axon_ntff_profile_hook = lambda: mod._hook
    mod.set_axon_ntff_profile_hook = lambda h: setattr(mod, "_hook", h)
    sys.modules[name] = mod


# ======================================================================
# Host preprocessing: SPMD token streams + per-core data
# ======================================================================

def preprocess(edge_index, n_nodes, ncores, lo_split):
    row = np.asarray(edge_index[0], dtype=np.int64)
    col = np.asarray(edge_index[1], dtype=np.int64)
    E = row.shape[0]
    r_per = n_nodes // ncores
    nwin = (r_per + 127) // 128

    deg = np.bincount(row, minlength=n_nodes).astype(np.float64)
    dinv = np.where(deg > 0, 1.0 / np.sqrt(np.maximum(deg, 1.0)), 0.0)
    escale_all = (dinv[row] * dinv[col]).astype(np.float32)

    core = row // r_per
    lrow = row - core * r_per
    win = lrow // 128
    is_hi = (col >= lo_split).astype(np.int64)

    # stream order: core, then stream (lo/hi), then window, then lrow
    order = np.lexsort((lrow, win, is_hi, core))
    core_s, win_s, hi_s = core[order], win[order], is_hi[order]
    lrow_s, col_s, esc_s = lrow[order], col[order], escale_all[order]

    key = (core_s * 2 + hi_s) * nwin + win_s
    cnt = np.bincount(key, minlength=ncores * 2 * nwin).reshape(ncores, 2, nwin)
    sec_len = ((cnt.max(axis=0) + 127) // 128) * 128  # [2, nwin]
    L_lo = int(sec_len[0].sum())
    L_hi = int(sec_len[1].sum())
    e_tok = L_lo + L_hi
    sec_start = np.zeros((2, nwin), np.int64)
    sec_start[0] = np.concatenate([[0], np.cumsum(sec_len[0])[:-1]])
    sec_start[1] = L_lo + np.concatenate([[0], np.cumsum(sec_len[1])[:-1]])

    col16 = np.zeros((ncores, e_tok), np.int16)
    rowrel = np.zeros((ncores, e_tok), np.float32)
    esc = np.zeros((ncores, e_tok), np.float32)

    grp_first = np.zeros(ncores * 2 * nwin + 1, np.int64)
    np.cumsum(cnt.reshape(-1), out=grp_first[1:])
    rank = np.arange(E) - grp_first[key]
    dest = sec_start[hi_s, win_s] + rank
    cval = np.where(hi_s == 1, col_s - lo_split, col_s).astype(np.int16)
    col16[core_s, dest] = cval
    rowrel[core_s, dest] = (lrow_s - win_s * 128).astype(np.float32)
    esc[core_s, dest] = esc_s

    # gather calls per stream
    calls = []  # (stream, ts, nt)
    for h, base, L in ((0, 0, L_lo), (1, L_lo, L_hi)):
        off = 0
        while off < L:
            nt = min(CALL_TOKENS, L - off)
            calls.append((h, base + off, nt))
            off += nt

    idx_dev = np.zeros((ncores, 128, e_tok // 16), np.int16)
    for (h, ts, nt) in calls:
        blk = col16[:, ts:ts + nt].reshape(ncores, nt // 16, 16)
        blk = np.ascontiguousarray(np.transpose(blk, (0, 2, 1)))
        idx_dev[:, :, ts // 16:(ts + nt) // 16] = np.tile(blk, (1, 8, 1))
    rr_dev = np.ascontiguousarray(rowrel.reshape(ncores, -1, 128).transpose(0, 2, 1))
    es_dev = np.ascontiguousarray(esc.reshape(ncores, -1, 128).transpose(0, 2, 1))

    return {
        "nwin": nwin, "e_tok": e_tok, "sec_len": sec_len, "calls": calls,
        "idx_dev": idx_dev, "rr_dev": rr_dev, "es_dev": es_dev,
    }


# ======================================================================
# Kernel builder
# ======================================================================

def build_kernel(meta, n_nodes, in_ch, hidden, out_ch, eps, lo_split, ncores):
    nwin = meta["nwin"]
    e_tok = meta["e_tok"]
    sec_len = meta["sec_len"]
    calls = meta["calls"]
    nchunk_tot = (n_nodes + 127) // 128
    r_per = n_nodes // ncores
    last_win_rows = r_per - 128 * (nwin - 1)
    kt = in_ch // 128
    hh = hidden // 2  # f32 slots holding the bf16 h vector

    # tile -> window map, and burst boundaries per (stream, window)
    tiles_w = []
    burst = {}  # (h, w) -> (gfirst, glast) in global tile idx
    for h in range(2):
        for w in range(nwin):
            ntl = int(sec_len[h, w]) // 128
            if ntl == 0:
                continue
            g0 = len(tiles_w)
            tiles_w.extend([w] * ntl)
            burst[(h, w)] = (g0, g0 + ntl - 1)
    assert len(tiles_w) == e_tok // 128
    last_stream = {}
    for w in range(nwin):
        last_stream[w] = 1 if (1, w) in burst else 0

    ngrp = nchunk_tot // PREP_GRP
    grp_rem = nchunk_tot - ngrp * PREP_GRP
    ogrp = nwin // PREP_GRP
    ogrp_rem = nwin - ogrp * PREP_GRP

    nc = bacc.Bacc("TRN2", target_bir_lowering=False, debug=False,
                   num_devices=ncores, num_swdge_queues=4)

    # ---- I/O ----
    # xtg: host-prearranged [group, 128p, grp*kt, 128] bf16 (+ ones row separately)
    xtg = nc.dram_tensor("xtg", [ngrp + (1 if grp_rem else 0), 128, PREP_GRP * kt, 128], BF16, kind="ExternalInput")
    xbg = nc.dram_tensor("xbg", [ngrp + (1 if grp_rem else 0), 1, PREP_GRP, 128], BF16, kind="ExternalInput")
    xtog = nc.dram_tensor("xtog", [ogrp + (1 if ogrp_rem else 0), 128, PREP_GRP * kt, 128], BF16, kind="ExternalInput")
    xbog = nc.dram_tensor("xbog", [ogrp + (1 if ogrp_rem else 0), 1, PREP_GRP, 128], BF16, kind="ExternalInput")
    t1wt = nc.dram_tensor("t1wt", [in_ch + 1, hidden], BF16, kind="ExternalInput")
    gwrep = nc.dram_tensor("gwrep", [4, 128, hidden], BF16, kind="ExternalInput")
    gbrep = nc.dram_tensor("gbrep", [128, 2], F32, kind="ExternalInput")
    t2wt = nc.dram_tensor("t2wt", [hidden, out_ch], F32, kind="ExternalInput")
    t2b = nc.dram_tensor("t2b", [1, out_ch], F32, kind="ExternalInput")
    iotac_in = nc.dram_tensor("iotac", [128, CT_MAX * 128], BF16, kind="ExternalInput")
    ident_in = nc.dram_tensor("ident", [128, 128], F32, kind="ExternalInput")
    identb_in = nc.dram_tensor("identb", [128, 128], BF16, kind="ExternalInput")
    ones_in = nc.dram_tensor("ones", [1, 128], F32, kind="ExternalInput")
    idx_in = nc.dram_tensor("idx", [128, e_tok // 16], I16, kind="ExternalInput")
    rrb_in = nc.dram_tensor("rrb", [128, e_tok // 128], BF16, kind="ExternalInput")
    es_in = nc.dram_tensor("es", [128, e_tok // 128], F32, kind="ExternalInput")
    out = nc.dram_tensor("out", [r_per, out_ch], F32, kind="ExternalOutput")

    ext0 = nc.dram_tensor("ext0", [nchunk_tot * 128, EXT_SLOTS], F32)
    agi = nc.dram_tensor("agi", [r_per, EXT_SLOTS], F32)
    ago = nc.dram_tensor("ago", [r_per * ncores, EXT_SLOTS], F32)

    with tile.TileContext(nc) as tc:
        nc.gpsimd.load_library(library_config.mlp)
        with tc.tile_pool(name="consts", bufs=1) as cp:
            t1wt_sb = cp.tile([128, kt, hidden], BF16, tag="t1wt")
            nc.sync.dma_start(t1wt_sb[:], bass.AP(t1wt, 0, [[hidden, 128], [128 * hidden, kt], [1, hidden]]))
            t1b_sb = cp.tile([1, hidden], BF16, tag="t1b")
            nc.sync.dma_start(t1b_sb[:], t1wt.ap()[in_ch:in_ch + 1, :])
            gw_sb = cp.tile([128, 4, hidden], BF16, tag="gw")
            nc.sync.dma_start(gw_sb[:], bass.AP(gwrep, 0, [[hidden, 128], [128 * hidden, 4], [1, hidden]]))
            gb_sb = cp.tile([128, 2], F32, tag="gb")
            nc.sync.dma_start(gb_sb[:], gbrep.ap())
            t2wt_sb = cp.tile([128, out_ch], F32, tag="t2wt")
            nc.sync.dma_start(t2wt_sb[:], t2wt.ap())
            t2b_sb = cp.tile([1, out_ch], F32, tag="t2b")
            nc.sync.dma_start(t2b_sb[:], t2b.ap())
            ident_sb = cp.tile([128, 128], F32, tag="ident")
            nc.sync.dma_start(ident_sb[:], ident_in.ap())
            identb_sb = cp.tile([128, 128], BF16, tag="identb")
            nc.sync.dma_start(identb_sb[:], identb_in.ap())
            ones_sb = cp.tile([1, 128], F32, tag="ones")
            nc.sync.dma_start(ones_sb[:], ones_in.ap())
            idxt = cp.tile([128, e_tok // 16], I16, tag="idxt")
            nc.sync.dma_start(idxt[:], idx_in.ap())
            rrb_sb = cp.tile([128, e_tok // 128], BF16, tag="rrb")
            nc.sync.dma_start(rrb_sb[:], rrb_in.ap())
            es_sb = cp.tile([128, e_tok // 128], F32, tag="es")
            nc.sync.dma_start(es_sb[:], es_in.ap())
            iotac_sb = cp.tile([128, CT_MAX, 128], BF16, tag="iotac")
            nc.sync.dma_start(iotac_sb[:], iotac_in.ap())

            rawsc = cp.tile([128, nwin, hidden], F32, tag="rawsc")
            acc = cp.tile([128, nwin, hidden], F32, tag="acc")
            a_arr = cp.tile([128, nwin, 2], BF16, tag="a_arr")

            # ---------------- prep: replicated gather table ----------------
            with tc.tile_pool(name="prep", bufs=3) as pp, \
                 tc.tile_pool(name="prep_s", bufs=6) as pscr, \
                 tc.tile_pool(name="prep_ps", bufs=4, space="PSUM") as pps:

                def prep_group(gi, gcnt, xt_t, xb_t, own):
                    xt_sb = pp.tile([128, PREP_GRP * kt, 128], BF16, tag="xt")
                    nc.sync.dma_start(xt_sb[:, 0:gcnt * kt, :], xt_t.ap()[gi, :, 0:gcnt * kt, :])
                    xb_sb = pp.tile([1, PREP_GRP, 128], BF16, tag="xb")
                    nc.sync.dma_start(xb_sb[:, 0:gcnt, :], xb_t.ap()[gi, :, 0:gcnt, :])
                    if not own:
                        extg = pp.tile([128, PREP_GRP, B_SLOT + 1], F32, tag="extg")
                    for c in range(gcnt):
                        ps = pps.tile([128, hidden], F32, tag="h0ps")
                        for k in range(kt):
                            nc.tensor.matmul(ps[:], xt_sb[:, c * kt + k, :], t1wt_sb[:, k, :],
                                             start=(k == 0), stop=False)
                        nc.tensor.matmul(ps[:], xb_sb[:, c, :], t1b_sb[:], start=False, stop=True)
                        if own:
                            w = gi * PREP_GRP + c
                            nc.vector.tensor_scalar(out=rawsc[:, w, :], in0=ps[:],
                                                    scalar1=0.0, scalar2=eps,
                                                    op0=mybir.AluOpType.max,
                                                    op1=mybir.AluOpType.mult)
                            hb = pscr.tile([128, hidden], BF16, tag="hb")
                            nc.scalar.activation(hb[:], ps[:], mybir.ActivationFunctionType.Relu)
                            scr = pscr.tile([128, hidden], BF16, tag="scr")
                            a_f = pscr.tile([128, 1], F32, tag="af")
                            nc.vector.scalar_tensor_tensor(
                                out=scr[:], in0=hb[:], scalar=1.0, in1=gw_sb[:, 0, :],
                                op0=mybir.AluOpType.mult, op1=mybir.AluOpType.mult,
                                accum_out=a_f[:])
                            nc.vector.tensor_scalar(out=a_arr[:, w, 0:1], in0=a_f[:],
                                                    scalar1=gb_sb[:, 0:1], scalar2=None,
                                                    op0=mybir.AluOpType.add)
                        else:
                            hb = extg[:, c, 0:hh].bitcast(BF16)
                            nc.scalar.activation(hb, ps[:], mybir.ActivationFunctionType.Relu)
                            scr = pscr.tile([128, hidden], BF16, tag="scr")
                            nc.vector.scalar_tensor_tensor(
                                out=scr[:], in0=hb, scalar=1.0, in1=gw_sb[:, 1, :],
                                op0=mybir.AluOpType.mult, op1=mybir.AluOpType.mult,
                                accum_out=extg[:, c, B_SLOT:B_SLOT + 1])
                    if not own:
                        base = gi * PREP_GRP * 128
                        nc.sync.dma_start(
                            bass.AP(ext0, base * EXT_SLOTS,
                                    [[EXT_SLOTS, 128], [128 * EXT_SLOTS, gcnt], [1, B_SLOT + 1]]),
                            extg[:, 0:gcnt, :])

                for gi in range(ngrp + (1 if grp_rem else 0)):
                    prep_group(gi, PREP_GRP if gi < ngrp else grp_rem, xtg, xbg, False)
                for gi in range(ogrp + (1 if ogrp_rem else 0)):
                    prep_group(gi, PREP_GRP if gi < ogrp else ogrp_rem, xtog, xbog, True)

            # ---------------- edge phase (per layer) ----------------
            def emit_layer(l, table):
                lo_ap = table.ap()
                hi_ap = table.ap()[lo_split:, :]
                awb = cp.tile([128, nwin, 128], BF16, tag=f"awb{l}")
                with tc.tile_pool(name=f"g{l}", bufs=int(os.environ.get("KGB", "4"))) as gp, \
                     tc.tile_pool(name=f"s{l}", bufs=3) as sp, \
                     tc.tile_pool(name=f"scr{l}", bufs=6) as scrp, \
                     tc.tile_pool(name=f"oh{l}", bufs=3) as ohp, \
                     tc.tile_pool(name=f"fin{l}", bufs=2) as fp, \
                     tc.tile_pool(name=f"aw{l}", bufs=4) as awp, \
                     tc.tile_pool(name=f"awps{l}", bufs=2, space="PSUM") as awps, \
                     tc.tile_pool(name=f"psW{l}", bufs=2, space="PSUM") as psW:
                    # per-window broadcast of the a-term: awb[p, w, f] = a_arr[f, w, l]
                    for w in range(nwin):
                        tp_ps = awps.tile([1, 128], F32, tag="aT")
                        nc.tensor.matmul(tp_ps[:], a_arr[:, w, l:l + 1], identb_sb[:],
                                         start=True, stop=True)
                        aT = awp.tile([1, 128], F32, tag="aTs")
                        nc.vector.tensor_copy(aT[:], tp_ps[:])
                        ab_ps = awps.tile([128, 128], F32, tag="ab")
                        nc.tensor.matmul(ab_ps[:], ones_sb[:], aT[:],
                                         start=True, stop=True)
                        nc.scalar.activation(awb[:, w, :], ab_ps[:],
                                             mybir.ActivationFunctionType.Copy)
                    def finalize(w):
                        rows = 128 if w < nwin - 1 else last_win_rows
                        if l == 0:
                            ext1 = fp.tile([128, B_SLOT + 1], F32, tag="ext1")
                            h1b = ext1[:, 0:hh].bitcast(BF16)
                            nc.vector.tensor_copy(h1b, acc[:, w, :])
                            scr = scrp.tile([128, hidden], BF16, tag="escr")
                            nc.vector.scalar_tensor_tensor(
                                out=scr[:], in0=h1b, scalar=1.0, in1=gw_sb[:, 3, :],
                                op0=mybir.AluOpType.mult, op1=mybir.AluOpType.mult,
                                accum_out=ext1[:, B_SLOT:B_SLOT + 1])
                            scr2 = scrp.tile([128, hidden], BF16, tag="escr2")
                            a_f = scrp.tile([128, 1], F32, tag="af1")
                            nc.vector.scalar_tensor_tensor(
                                out=scr2[:], in0=h1b, scalar=1.0, in1=gw_sb[:, 2, :],
                                op0=mybir.AluOpType.mult, op1=mybir.AluOpType.mult,
                                accum_out=a_f[:])
                            nc.vector.tensor_scalar(out=a_arr[:, w, 1:2], in0=a_f[:],
                                                    scalar1=gb_sb[:, 1:2], scalar2=None,
                                                    op0=mybir.AluOpType.add)
                            nc.sync.dma_start(agi.ap()[w * 128:w * 128 + rows, 0:B_SLOT + 1],
                                              ext1[0:rows, :])

                    # windows with no lo-burst: seed acc with rawsc; fully
                    # edgeless windows also finalize immediately
                    for w in range(nwin):
                        if (0, w) not in burst:
                            nc.vector.tensor_copy(acc[:, w, :], rawsc[:, w, :])
                            if (1, w) not in burst:
                                finalize(w)

                    qi = 0
                    W_ps = None
                    for (h, ts, nt) in [c for c in calls]:
                        ct = nt // 128
                        t0 = ts // 128
                        G = gp.tile([128, CT_MAX, EXT_SLOTS], F32, tag="G")
                        nc.gpsimd.dma_gather(
                            out_ap=G[:, 0:ct, :],
                            in_ap=(hi_ap if h else lo_ap),
                            idxs_ap=idxt[:, ts // 16:(ts + nt) // 16],
                            num_idxs=nt, num_idxs_reg=nt, elem_size=EXT_SLOTS,
                            single_packet=False, queue_num=qi % 4)
                        qi += 1
                        # split call into window-pure runs
                        runs = []
                        c = 0
                        while c < ct:
                            w = tiles_w[t0 + c]
                            c1 = c
                            while c1 < ct and tiles_w[t0 + c1] == w:
                                c1 += 1
                            runs.append((w, c, c1))
                            c = c1
                        # batched one-hot equality: eqc[p, c, f] = (iota[f]==rr[p,c])
                        eqc = ohp.tile([128, CT_MAX, 128], BF16, tag="eqc")
                        rrs = rrb_sb[:, t0:t0 + ct]
                        rrx = bass.AP(rrs.tensor, rrs.offset, list(rrs.ap) + [[0, 128]])
                        nc.vector.tensor_tensor(out=eqc[:, 0:ct, :],
                                                in0=iotac_sb[:, 0:ct, :], in1=rrx,
                                                op=mybir.AluOpType.is_equal)
                        # pass 1: a-term selected from awb via the one-hot
                        atk = sp.tile([128, CT_MAX], F32, tag="atk")
                        for (w, c0, c1) in runs:
                            rK = c1 - c0
                            aws = awb[:, w, :]
                            awx = bass.AP(aws.tensor, aws.offset,
                                          [list(aws.ap)[0], [0, rK], list(aws.ap)[1]])
                            tmpa = scrp.tile([128, CT_MAX, 128], BF16, tag="tmpa")
                            nc.vector.tensor_tensor(out=tmpa[:, 0:rK, :],
                                                    in0=eqc[:, c0:c1, :], in1=awx,
                                                    op=mybir.AluOpType.mult)
                            nc.vector.reduce_sum(out=atk[:, c0:c1],
                                                 in_=tmpa[:, 0:rK, :],
                                                 axis=mybir.AxisListType.X)
                        arg = sp.tile([128, CT_MAX], F32, tag="arg")
                        nc.vector.tensor_tensor(out=arg[:, 0:ct], in0=atk[:, 0:ct],
                                                in1=G[:, 0:ct, B_SLOT],
                                                op=mybir.AluOpType.add)
                        gt = sp.tile([128, CT_MAX], BF16, tag="gt")
                        nc.scalar.activation(gt[:, 0:ct], arg[:, 0:ct],
                                             mybir.ActivationFunctionType.Tanh)
                        wt = sp.tile([128, CT_MAX], BF16, tag="wt")
                        nc.vector.tensor_tensor(out=wt[:, 0:ct], in0=gt[:, 0:ct],
                                                in1=es_sb[:, t0:t0 + ct],
                                                op=mybir.AluOpType.mult)
                        ohc = ohp.tile([128, CT_MAX, 128], BF16, tag="ohc")
                        wts = wt[:, 0:ct]
                        wtx = bass.AP(wts.tensor, wts.offset, list(wts.ap) + [[0, 128]])
                        nc.vector.tensor_tensor(out=ohc[:, 0:ct, :],
                                                in0=eqc[:, 0:ct, :], in1=wtx,
                                                op=mybir.AluOpType.mult)
                        # pass 2: one-hot scatter matmuls
                        for (w, c0, c1) in runs:
                            bf, bl = burst[(h, w)]
                            if t0 + c0 == bf:
                                W_ps = psW.tile([128, hidden], F32, tag="W")
                            for c in range(c0, c1):
                                nc.tensor.matmul(W_ps[:], ohc[:, c, :], G[:, c, 0:hh].bitcast(BF16),
                                                 start=(t0 + c == bf),
                                                 stop=(t0 + c == bl))
                            if t0 + c1 - 1 == bl:
                                if h == 0:
                                    nc.vector.tensor_tensor(out=acc[:, w, :], in0=W_ps[:],
                                                            in1=rawsc[:, w, :],
                                                            op=mybir.AluOpType.add)
                                    if last_stream[w] == 0:
                                        finalize(w)
                                else:
                                    nc.vector.tensor_tensor(out=acc[:, w, :], in0=W_ps[:],
                                                            in1=acc[:, w, :],
                                                            op=mybir.AluOpType.add)
                                    finalize(w)

            phase = os.environ.get("KPHASE", "head")
            plvl = {"prep": 0, "l0": 1, "cc": 2, "l1": 3, "head": 4}[phase]
            if plvl >= 1:
                emit_layer(0, ext0)
            if plvl >= 2:
                nc.gpsimd.collective_compute(
                    "AllGather", mybir.AluOpType.bypass,
                    replica_groups=[list(range(ncores))],
                    ins=[agi.ap().opt()], outs=[ago.ap().opt()])
            if plvl >= 3:
                emit_layer(1, ago)
            if plvl < 4:
                with tc.tile_pool(name="zout", bufs=1) as zp:
                    o_z = zp.tile([128, out_ch], F32, tag="oz")
                    nc.vector.memset(o_z[:], 0.0)
                    for w in range(nwin):
                        rows = 128 if w < nwin - 1 else last_win_rows
                        nc.sync.dma_start(out.ap()[w * 128:w * 128 + rows, :],
                                          o_z[0:rows, :])
                return nc

            # ---------------- head: out = log_softmax(h @ t2^T + b) ----------
            # two passes so the Act engine loads the Exp/Ln tables once each
            with tc.tile_pool(name="head", bufs=4) as hp, \
                 tc.tile_pool(name="head_ps", bufs=4, space="PSUM") as hps:
                o_all = cp.tile([128, nwin, out_ch], F32, tag="o_all")
                nm_all = cp.tile([128, nwin], F32, tag="nm_all")
                s_all = cp.tile([128, nwin], F32, tag="s_all")
                for w in range(nwin):
                    ht_ps = hps.tile([128, 128], F32, tag="ht")
                    nc.tensor.matmul(ht_ps[:], acc[:, w, :], ident_sb[:],
                                     start=True, stop=True)
                    ht_sb = hp.tile([128, 128], F32, tag="ht_sb")
                    nc.vector.tensor_copy(ht_sb[:], ht_ps[:])
                    o_ps = hps.tile([128, out_ch], F32, tag="ops")
                    nc.tensor.matmul(o_ps[:], ht_sb[:], t2wt_sb[:], start=True, stop=False)
                    nc.tensor.matmul(o_ps[:], ones_sb[:], t2b_sb[:], start=False, stop=True)
                    nc.vector.reduce_max(out=nm_all[:, w:w + 1], in_=o_ps[:],
                                         axis=mybir.AxisListType.X, negate=True)
                    e_sb = hp.tile([128, out_ch], F32, tag="e")
                    nc.scalar.activation(e_sb[:], o_ps[:],
                                         mybir.ActivationFunctionType.Exp,
                                         bias=nm_all[:, w:w + 1])
                    nc.vector.reduce_sum(out=s_all[:, w:w + 1], in_=e_sb[:],
                                         axis=mybir.AxisListType.X)
                    nc.vector.tensor_copy(o_all[:, w, :], o_ps[:])
                ls_all = cp.tile([128, nwin], F32, tag="ls_all")
                nc.scalar.activation(ls_all[:], s_all[:], mybir.ActivationFunctionType.Ln)
                for w in range(nwin):
                    rows = 128 if w < nwin - 1 else last_win_rows
                    o_sb = hp.tile([128, out_ch], F32, tag="o")
                    nc.vector.tensor_scalar(out=o_sb[:], in0=o_all[:, w, :],
                                            scalar1=nm_all[:, w:w + 1],
                                            scalar2=ls_all[:, w:w + 1],
                                            op0=mybir.AluOpType.add,
                                            op1=mybir.AluOpType.subtract)
                    nc.sync.dma_start(out.ap()[w * 128:w * 128 + rows, :], o_sb[0:rows, :])

    return nc


# ======================================================================
# Host driver
# ======================================================================

def _bf16(a):
    import ml_dtypes
    return np.asarray(a, dtype=ml_dtypes.bfloat16)


def _group_x(xT_pad, nrow_units, kt):
    # xT_pad: [in_ch+1, units*128] f32 -> xtg [ngrp, 128, PREP_GRP*kt, 128],
    # xbg [ngrp, 1, PREP_GRP, 128] (ones row)
    in_ch = (xT_pad.shape[0] - 1)
    ngrp_t = (nrow_units + PREP_GRP - 1) // PREP_GRP
    pad_units = ngrp_t * PREP_GRP
    xp = np.zeros((in_ch + 1, pad_units * 128), np.float32)
    xp[:, :xT_pad.shape[1]] = xT_pad
    # [in, u, 128] -> [u, in, 128]
    xr = xp[:in_ch].reshape(in_ch, pad_units, 128).transpose(1, 0, 2)
    # [g, c, k, p, r] with in = k*128+p
    xg = xr.reshape(ngrp_t, PREP_GRP, kt, 128, 128)
    xtg = np.ascontiguousarray(xg.transpose(0, 3, 1, 2, 4)).reshape(
        ngrp_t, 128, PREP_GRP * kt, 128)
    xb = xp[in_ch].reshape(ngrp_t, 1, PREP_GRP, 128)
    return _bf16(xtg), _bf16(np.ascontiguousarray(xb))


def kernel_run(x, edge_index, t1_w, t1_b, gate_w, gate_b, t2_w, t2_b,
               n_nodes=N_NODES, in_ch=IN_CH, hidden=HIDDEN, out_ch=OUT_CH,
               eps=EPS, ncores=NCORES, lo_split=None, trace=False):
    _install_profile_hook()
    from concourse import bass_utils

    if lo_split is None:
        lo_split = min(25000, ((n_nodes + 1) // 2 + 127) // 128 * 128)
    meta = preprocess(edge_index, n_nodes, ncores, lo_split)
    nwin = meta["nwin"]
    r_per = n_nodes // ncores
    nchunk_tot = (n_nodes + 127) // 128
    kt = in_ch // 128

    nc = build_kernel(meta, n_nodes, in_ch, hidden, out_ch, eps, lo_split, ncores)
    nc.finalize()

    # host arrays
    x = np.asarray(x, np.float32)
    xT = np.concatenate([x.T, np.ones((1, x.shape[0]), np.float32)], axis=0)  # [in+1, N]
    pad_n = nchunk_tot * 128
    xT_pad = np.zeros((in_ch + 1, pad_n), np.float32)
    xT_pad[:, :n_nodes] = xT
    xtg_h, xbg_h = _group_x(xT_pad, nchunk_tot, kt)

    t1wt_h = _bf16(np.concatenate([np.asarray(t1_w, np.float32).T,
                                   np.asarray(t1_b, np.float32)[None, :]], axis=0))
    gw = np.asarray(gate_w, np.float32)
    gwrep_h = _bf16(np.stack([
        np.tile(gw[0, :hidden][None, :], (128, 1)),
        np.tile(gw[0, hidden:][None, :], (128, 1)),
        np.tile(gw[1, :hidden][None, :], (128, 1)),
        np.tile(gw[1, hidden:][None, :], (128, 1))]))
    gbrep_h = np.tile(np.asarray(gate_b, np.float32)[None, :], (128, 1))
    t2wt_h = np.ascontiguousarray(np.asarray(t2_w, np.float32).T)
    t2b_h = np.asarray(t2_b, np.float32)[None, :]
    iotac_h = _bf16(np.tile(np.arange(128, dtype=np.float32)[None, :],
                            (128, CT_MAX)))
    ident_h = np.eye(128, dtype=np.float32)
    identb_h = _bf16(np.eye(128, dtype=np.float32))
    ones_h = np.ones((1, 128), np.float32)

    in_maps = []
    for c in range(ncores):
        sl = np.zeros((in_ch + 1, nwin * 128), np.float32)
        take = min(nwin * 128, xT.shape[1] - c * r_per)
        sl[:, :take] = xT[:, c * r_per: c * r_per + take]
        xtog_h, xbog_h = _group_x(sl, nwin, kt)
        in_maps.append({
            "xtg": xtg_h, "xbg": xbg_h, "xtog": xtog_h, "xbog": xbog_h,
            "t1wt": t1wt_h, "gwrep": gwrep_h, "gbrep": gbrep_h,
            "t2wt": t2wt_h, "t2b": t2b_h,
            "iotac": iotac_h, "ident": ident_h, "identb": identb_h,
            "ones": ones_h,
            "idx": meta["idx_dev"][c],
            "rrb": _bf16(meta["rr_dev"][c]),
            "es": meta["es_dev"][c],
        })

    res = bass_utils.run_bass_kernel_spmd(
        nc, in_maps, core_ids=list(range(ncores)), trace=trace)
    outp = np.concatenate([res.results[c]["out"] for c in range(ncores)], axis=0)
    return outp[:n_nodes], res


def kernel(**inputs):
    x = inputs["x"]
    edge_index = inputs["edge_index"]
    outp, _ = kernel_run(
        x, edge_index, inputs["t1_w"], inputs["t1_b"], inputs["gate_w"],
        inputs["gate_b"], inputs["t2_w"], inputs["t2_b"])
    return np.asarray(outp, np.float32)

